# revision 74
# baseline (speedup 1.0000x reference)
"""Trainium2 Bass kernel for nn_AudioNetwork (4-block STFT resonator chain).

Algorithm notes
---------------
Per block: frame x (win 2048, hop 1024), rfft, per-bin linear recurrence over
frames out_i = (spec_i + out_{i-1}) * tc, irfft, hann-windowed overlap-add,
tanh(gain * s).  Since every recurrence step multiplies by tc, bins with
tc == 0 never contribute: the (i)DFT only needs the nonzero bins of tc
(~10 of 1025 for the reference init).  Both transforms become tiny matmuls.

Device layout (per core, 4 batch elements), v3 (fp16 pipeline):
  The signal lives in SBUF as fp16 tiles (128 samples-in-chunk, KT, 1024 cols)
  where col = batch*256 + chunk.  Layout changes use the DMA crossbar
  transpose (dma_start_transpose, 16x128 xbar tiles) instead of PE
  transposes, so the PE only runs the DFT matmuls.  All crossbar transposes
  ride ONE queue: concurrent xbar DMAs from two queues clobber adjacent
  destination slices (observed as nondeterministic corruption on HW).
  HBM->SBUF input loads are gpsimd (software DGE) DMAs casting fp32->fp16
  in flight.

  Forward: spec_i needs frame i = [chunk_i, chunk_{i+1}] but
  cos/sin(2*pi*k*(s+1024)/2048) = (-1)^k * cos/sin(2*pi*k*s/2048), so only
  the half-window matrix U is computed; the second half is sign * U shifted
  by one frame.  The forward runs column-group-first (batches 0-1 then 2-3)
  so the recurrence work starts while the PE is still on batches 2-3.  The
  scan state is fp32 internally regardless of operand dtype, so the fp16
  recurrence does not accumulate rounding; tr stays fp32 since a 2^-11
  error in tc compounds over 256 frames.  The inverse needs the stack
  [out_cur; out_prev]: the scan writes rows 0:64 of scat at col k+1, and a
  GpSimd cross-partition copy fills rows 64:128 one column later (the
  leading memset zero provides the overlap-add edge), giving the inverse a
  single 128-row stationary with one uniform column offset.

  The inverse DFT matrices are scaled by 2**10 (compensated in the tanh
  scale) so the hann-window tails stay in fp16 normal range.  When the
  mixer weights are all equal (softmax of the zero mixer), the accumulator
  is plain adds (split DVE/GpSimd) with the weight folded into the store
  cast; accumulator updates run one block late so they never compete with
  the combine/scan chain.
"""

import numpy as np
from contextlib import ExitStack

import concourse.bass as bass
import concourse.tile as tile
from concourse import bacc, mybir
from concourse import bass_utils

F32 = mybir.dt.float32
F16 = mybir.dt.float16
WS = 2048
STEP = 1024
NCOEF = WS // 2 + 1
NBLK = 4
B = 32
T = 262144
NCORES = 8
BL = B // NCORES          # batch per core
NF = T // STEP            # 256 frames/chunks
KT = STEP // 128          # 8 K-tiles of the forward contraction
COLS = BL * NF            # 1024 free columns (batch-major)
MAX_BINS_PER_CHUNK = 32   # 2*nb must fit in a 64-row half
WI_SCALE = 1024.0         # keeps hann tails in fp16 normal range

_CACHE = {}


def _plan_chunks(tc_vec):
    nz = np.nonzero(tc_vec)[0]
    if len(nz) == 0:
        nz = np.array([1], dtype=np.int64)  # dummy bin with tc=0: contributes 0
    chunks = [nz[i:i + MAX_BINS_PER_CHUNK] for i in range(0, len(nz), MAX_BINS_PER_CHUNK)]
    return chunks


def _host_matrices(tc_vec, chunks):
    """Build per-chunk constant arrays (float64 math, fp16/fp32 storage)."""
    hann = 0.5 - 0.5 * np.cos(2.0 * np.pi * np.arange(WS) / WS)
    out = []
    for bins in chunks:
        nb = len(bins)
        k = bins.astype(np.float64)
        tcv = tc_vec[bins].astype(np.float64)
        s = np.arange(STEP, dtype=np.float64)
        ang = 2.0 * np.pi * np.outer(s, k) / WS                      # (1024, nb)
        # duplicated on both column halves: the matmul then writes U to
        # partitions 0:64 and 64:128 at no extra PE cost, keeping both scans
        # partition-aligned
        bf = np.zeros((STEP, 128))
        bf[:, 0:nb] = np.cos(ang) * tcv
        bf[:, nb:2 * nb] = -np.sin(ang) * tcv
        bf[:, 64:64 + 2 * nb] = bf[:, 0:2 * nb]
        bf_t = bf.reshape(KT, 128, 128).transpose(1, 0, 2)           # (128, 8, 128)
        sign = np.zeros((128, 1))
        sign[0:nb, 0] = (-1.0) ** k
        sign[nb:2 * nb, 0] = (-1.0) ** k
        sign[64:64 + 2 * nb] = sign[0:2 * nb]
        tcrep = np.zeros((128, NF))
        tcrep[0:nb] = tcv[:, None]
        tcrep[nb:2 * nb] = tcv[:, None]
        tcrep[64:64 + 2 * nb] = tcrep[0:2 * nb]
        w = np.where((bins == 0) | (bins == WS // 2), 1.0, 2.0)
        s2 = np.arange(WS, dtype=np.float64)
        ang2 = 2.0 * np.pi * np.outer(k, s2) / WS                    # (nb, 2048)
        are = (w[:, None] / WS) * np.cos(ang2) * hann * WI_SCALE
        aim = -(w[:, None] / WS) * np.sin(ang2) * hann * WI_SCALE
        w1 = np.concatenate([are[:, :STEP], aim[:, :STEP]], axis=0)  # cur frame
        w2 = np.concatenate([are[:, STEP:], aim[:, STEP:]], axis=0)  # prev frame
        pad = np.zeros((64 - 2 * nb, WS // 2))
        winv = np.concatenate([w1, pad, w2, pad], axis=0).reshape(128, KT, 128)
        out.append(dict(
            nb=nb,
            bf=np.ascontiguousarray(bf_t, dtype=np.float16),
            winv=np.ascontiguousarray(winv, dtype=np.float16),
            sign=np.ascontiguousarray(sign, dtype=np.float16),
            tcrep=np.ascontiguousarray(tcrep, dtype=np.float32),
        ))
    return out


def _build(chunk_sizes, gains, wmix, debug_taps=False):
    """Trace+compile the Bass program. chunk_sizes: tuple of tuples of nb per block."""
    nc = bacc.Bacc("TRN2", target_bir_lowering=False, debug=False)
    # x arrives host-pre-transposed to the device layout (sample-in-chunk on
    # partitions, k-tile, batch*frame columns) in fp16; the output is the
    # fp16 accumulator in the same layout, un-transposed and scaled on host
    x_d = nc.dram_tensor("x", (128, KT, COLS), F16, kind="ExternalInput").ap()
    out_d = nc.dram_tensor("out", (128, KT, COLS), F16, kind="ExternalOutput").ap()
    taps = {}
    if debug_taps:
        for nm, shp in [("t_x16", (128, KT, COLS)), ("t_dst1", (128, KT, COLS)),
                        ("t_scat1", (128, BL, NF + 1)), ("t_acc", (128, KT, COLS))]:
            taps[nm] = nc.dram_tensor(nm, shp, F16, kind="ExternalOutput").ap()
    # consolidated constants: one (bf, wi) pair for block 1, one blob for the
    # rest, plus all-sign/all-tr blobs — 6 input DMAs instead of 16
    chunks_flat = [(kb, c) for kb in range(NBLK) for c in range(len(chunk_sizes[kb]))]
    nch_tot = len(chunks_flat)
    nch_0 = len(chunk_sizes[0])
    nch_r = nch_tot - nch_0
    cons = {
        "bf0": nc.dram_tensor("bf0", (128, KT, 128 * nch_0), F16, kind="ExternalInput").ap(),
        "wi0": nc.dram_tensor("wi0", (128, KT, 128 * nch_0), F16, kind="ExternalInput").ap(),
        "sgA": nc.dram_tensor("sgA", (128, nch_tot), F16, kind="ExternalInput").ap(),
        "trA": nc.dram_tensor("trA", (128, NF * nch_tot), F32, kind="ExternalInput").ap(),
    }
    if nch_r:
        cons["bfR"] = nc.dram_tensor("bfR", (128, KT, 128 * nch_r), F16, kind="ExternalInput").ap()
        cons["wiR"] = nc.dram_tensor("wiR", (128, KT, 128 * nch_r), F16, kind="ExternalInput").ap()

    mult = mybir.AluOpType.mult
    add = mybir.AluOpType.add
    Tanh = mybir.ActivationFunctionType.Tanh
    Copy = mybir.ActivationFunctionType.Copy

    w_equal = bool(np.allclose(wmix, wmix[0], rtol=1e-7, atol=0.0))

    with tile.TileContext(nc) as tc, ExitStack() as ctx:
        cpool = ctx.enter_context(tc.tile_pool(name="const", bufs=1))
        big = ctx.enter_context(tc.tile_pool(name="big", bufs=1))
        work = ctx.enter_context(tc.tile_pool(name="work", bufs=2))
        # PSUM budget (8 banks): uv (2 banks) x2 + ips (1 bank) x4 = 8
        pmm = ctx.enter_context(tc.tile_pool(name="pmm", bufs=2, space="PSUM"))
        pmi = ctx.enter_context(tc.tile_pool(name="pmi", bufs=4, space="PSUM"))

        # consolidated resident constants; block 1's bf rides first on sync,
        # wi0 on scalar, interleaved with the x k-tile slab DMAs
        bf0_t = cpool.tile([128, KT, 128 * nch_0], F16, tag="bf0", name="bf0_t")
        nc.sync.dma_start(bf0_t[:], cons["bf0"][:])
        wi0_t = cpool.tile([128, KT, 128 * nch_0], F16, tag="wi0", name="wi0_t")
        nc.scalar.dma_start(wi0_t[:], cons["wi0"][:])
        sgA_t = cpool.tile([128, nch_tot], F16, tag="sgA", name="sgA_t")
        trA_t = cpool.tile([128, NF * nch_tot], F32, tag="trA", name="trA_t")
        bfR_t = wiR_t = None
        if nch_r:
            bfR_t = cpool.tile([128, KT, 128 * nch_r], F16, tag="bfR", name="bfR_t")
            wiR_t = cpool.tile([128, KT, 128 * nch_r], F16, tag="wiR", name="wiR_t")

        def bf_ap(kb, c):
            i = chunks_flat.index((kb, c))
            if kb == 0:
                return bf0_t[:, :, 128 * i:128 * (i + 1)]
            return bfR_t[:, :, 128 * (i - nch_0):128 * (i - nch_0 + 1)]

        def wi_ap(kb, c):
            i = chunks_flat.index((kb, c))
            if kb == 0:
                return wi0_t[:, :, 128 * i:128 * (i + 1)]
            return wiR_t[:, :, 128 * (i - nch_0):128 * (i - nch_0 + 1)]

        xbuf = [big.tile([128, KT, COLS], F16, tag=f"xb{i}", name=f"xb{i}") for i in range(2)]
        accb = big.tile([128, KT, COLS], F16, tag="acc", name="acc")
        # scat col k: rows 0:64 = out_{k-1} (scan), rows 64:128 = out_{k-2}
        # (cross-partition shifted copy); col 0 zero feeds the overlap edge
        scat = cpool.tile([128, BL, NF + 1], F16, tag="scat", name="scat")
        nc.vector.memset(scat[:, :, 0:2], 0.0)
        # uvs col NF stays zero: the sign-combine then covers all 256 cols
        uvs = cpool.tile([128, BL, NF + 1], F16, tag="uvs", name="uvs")
        nc.vector.memset(uvs[:, :, NF:NF + 1], 0.0)

        # ---- load x: one DMA per k-tile slab, alternating the two HWDGE
        # rings; the forward's a-th matmul fires as soon as slab a lands ----
        for m in range(KT):
            dma_eng = nc.sync if m % 2 == 0 else nc.scalar
            dma_eng.dma_start(xbuf[0][:, m, :], x_d[:, m, :])
        nc.gpsimd.dma_start(sgA_t[:], cons["sgA"][:])
        nc.sync.dma_start(trA_t[:], cons["trA"][:])
        if nch_r:
            nc.scalar.dma_start(bfR_t[:], cons["bfR"][:])
            nc.scalar.dma_start(wiR_t[:], cons["wiR"][:])
        for m in range(KT):
            if w_equal:
                nc.gpsimd.tensor_copy(accb[:, m, :], xbuf[0][:, m, :])
            else:
                nc.gpsimd.tensor_scalar_mul(accb[:, m, :], xbuf[0][:, m, :],
                                            float(wmix[0]))

        if debug_taps:
            nc.sync.dma_start(taps["t_x16"][:], xbuf[0][:])

        pending_acc = []

        def acc_update(m, t, w, cols, pool=False):
            if w_equal and pool:
                nc.gpsimd.tensor_tensor(accb[:, m, cols], accb[:, m, cols],
                                        t[:, m, cols], op=add)
            elif w_equal:
                nc.vector.tensor_tensor(accb[:, m, cols], accb[:, m, cols],
                                        t[:, m, cols], op=add)
            else:
                nc.vector.scalar_tensor_tensor(
                    accb[:, m, cols], t[:, m, cols], w, accb[:, m, cols], op0=mult, op1=add)

        def flush_acc():
            for m, t, w in pending_acc:
                acc_update(m, t, w, slice(None), pool=(m % 2 == 1))
            pending_acc.clear()

        # ---- block chain ----
        for kb in range(NBLK):
            if debug_taps and kb == 1:
                nc.sync.dma_start(taps["t_dst1"][:], xbuf[1][:])
            src = xbuf[kb % 2]
            dst = xbuf[(kb + 1) % 2]
            sizes = chunk_sizes[kb]
            nch = len(sizes)
            inv_sb = None
            if nch > 1:
                inv_sb = big.tile([128, KT, COLS], F32, tag="is", name=f"is{kb}")
            for c, nb in enumerate(sizes):
                ci = chunks_flat.index((kb, c))
                bf = bf_ap(kb, c)
                wi = wi_ap(kb, c)
                sg = sgA_t[:, ci:ci + 1]
                tr = trA_t[:, NF * ci:NF * (ci + 1)]
                gain = float(gains[kb]) / WI_SCALE

                # two independent column-group streams (batches 0-1 / 2-3):
                # PE runs fwd g0, fwd g1, inv g0, inv g1 back to back; the
                # recurrence for each group overlaps the PE work of the other,
                # and the next block's fwd g0 only waits on this block's g0
                # tanh halves
                def combine(b):
                    nc.vector.tensor_copy(uvs[:, b, 0:NF], uv[b // 2][:, b % 2, :])
                    in1 = work.tile([128, NF], F16, tag="in1", name=f"in1_{kb}_{c}_{b}")
                    nc.vector.scalar_tensor_tensor(
                        in1[:], uvs[:, b, 1:NF + 1], sg,
                        uvs[:, b, 0:NF], op0=mult, op1=add)
                    nc.vector.tensor_tensor_scan(
                        scat[0:64, b, 1:NF + 1], tr[0:64, :], in1[0:64, :],
                        initial=0.0, op0=mult, op1=add)
                    nc.vector.tensor_copy(scat[64:128, b, 1:NF + 1],
                                          scat[0:64, b, 0:NF])

                uv = [None, None]
                for g in range(2):
                    uv[g] = pmm.tile([128, 2, NF], F32, tag="uv", name=f"uv{kb}_{c}_{g}")
                    for a in range(KT):
                        nc.tensor.matmul(uv[g][:], bf[:, a, :],
                                         src[:, a, g * 512:(g + 1) * 512],
                                         start=(a == 0), stop=(a == KT - 1))
                    combine(2 * g)
                    combine(2 * g + 1)
                if debug_taps and kb == 0:
                    nc.scalar.dma_start(taps["t_scat1"][:], scat[:])
                flush_acc()
                last = kb == NBLK - 1 and c == nch - 1

                def inv_mm(g, m):
                    ip = pmi.tile([128, 512], F32, tag="ips", name=f"ip{kb}_{c}_{g}_{m}")
                    nc.tensor.matmul(ip[:], wi[:, m, :],
                                     scat[:, 2 * g:2 * g + 2, 1:NF + 1],
                                     start=True, stop=True)
                    if nch == 1:
                        return ip, dst[:, m, g * 512:(g + 1) * 512]
                    half = inv_sb[:, m, g * 512:(g + 1) * 512]
                    if c == 0:
                        nc.vector.tensor_copy(half, ip[:])
                    else:
                        nc.vector.tensor_add(half, half, ip[:])
                    return ip, half

                for m in range(KT):
                    ip0, h0 = inv_mm(0, m)
                    if nch == 1:
                        nc.scalar.activation(h0, ip0[:], Tanh, scale=gain)
                for m in range(KT):
                    ip1, h1 = inv_mm(1, m)
                    if nch == 1:
                        nc.scalar.activation(h1, ip1[:], Tanh, scale=gain)
                    elif c == nch - 1:
                        nc.scalar.activation(dst[:, m, :], inv_sb[:, m, :], Tanh,
                                             scale=gain)
                    if c == nch - 1:
                        if last:
                            # final block: accumulate and DMA this m-tile out
                            # right away so the store streams while the
                            # remaining inverse columns still compute
                            acc_update(m, dst, float(wmix[kb + 1]), slice(None),
                                       pool=(m in (1, 4)))
                            dma_eng = nc.sync if m % 2 == 0 else nc.scalar
                            dma_eng.dma_start(out_d[:, m, :], accb[:, m, :])
                        else:
                            pending_acc.append((m, dst, float(wmix[kb + 1])))

        # stores already streamed out inside block 4
        pending_acc.clear()
        if debug_taps:
            nc.sync.dma_start(taps["t_acc"][:], accb[:])

    nc.compile()
    return nc


def _const_map(transfers, plans):
    bfs, wis, sgs, trs, nch0 = [], [], [], [], None
    for kb in range(NBLK):
        mats = _host_matrices(transfers[kb].astype(np.float64), plans[kb])
        if kb == 0:
            nch0 = len(mats)
        for md in mats:
            bfs.append(md["bf"])
            wis.append(md["winv"])
            sgs.append(md["sign"])
            trs.append(md["tcrep"])
    cm = {
        "bf0": np.ascontiguousarray(np.concatenate(bfs[:nch0], axis=2)),
        "wi0": np.ascontiguousarray(np.concatenate(wis[:nch0], axis=2)),
        "sgA": np.ascontiguousarray(np.concatenate(sgs, axis=1)),
        "trA": np.ascontiguousarray(np.concatenate(trs, axis=1)),
    }
    if len(bfs) > nch0:
        cm["bfR"] = np.ascontiguousarray(np.concatenate(bfs[nch0:], axis=2))
        cm["wiR"] = np.ascontiguousarray(np.concatenate(wis[nch0:], axis=2))
    return cm


def _in_maps(x, const_map):
    # host-side shard + transpose to the device layout:
    # x16t[p, m, b*NF + f] = x[b, f*1024 + m*128 + p]
    xr = np.asarray(x, dtype=np.float32).reshape(B, T).astype(np.float16)
    maps = []
    for core in range(NCORES):
        xc = xr[core * BL:(core + 1) * BL].reshape(BL, NF, KT, 128)
        m = dict(const_map)
        m["x"] = np.ascontiguousarray(xc.transpose(3, 2, 0, 1).reshape(128, KT, COLS))
        maps.append(m)
    return maps


def _out_gather(res, scale):
    outs = []
    for i in range(NCORES):
        ot = res.results[i]["out"].reshape(128, KT, BL, NF)
        oc = ot.transpose(2, 3, 1, 0).reshape(BL, 1, T)
        outs.append(oc)
    return (np.concatenate(outs, axis=0).astype(np.float32) * np.float32(scale))


def kernel(x, transfers, gains, mixer):
    transfers = np.asarray(transfers, dtype=np.float32)
    gains = np.asarray(gains, dtype=np.float64)
    mixer = np.asarray(mixer, dtype=np.float64)
    wm = np.exp(mixer - mixer.max())
    wm = wm / wm.sum()

    plans = [_plan_chunks(transfers[kb]) for kb in range(NBLK)]
    chunk_sizes = tuple(tuple(len(ch) for ch in pl) for pl in plans)
    key = (chunk_sizes, tuple(np.round(gains, 9)), tuple(np.round(wm, 9)))
    if key not in _CACHE:
        _CACHE[key] = _build(chunk_sizes, gains, wm)
    nc = _CACHE[key]

    in_maps = _in_maps(x, _const_map(transfers, plans))
    res = bass_utils.run_bass_kernel_spmd(nc, in_maps, core_ids=list(range(NCORES)))
    w_equal = bool(np.allclose(wm, wm[0], rtol=1e-7, atol=0.0))
    return _out_gather(res, wm[0] if w_equal else 1.0)


# revision 77
# speedup vs baseline: 1.0146x; 1.0146x over previous
"""Trainium2 Bass kernel for nn_AudioNetwork (4-block STFT resonator chain).

Algorithm notes
---------------
Per block: frame x (win 2048, hop 1024), rfft, per-bin linear recurrence over
frames out_i = (spec_i + out_{i-1}) * tc, irfft, hann-windowed overlap-add,
tanh(gain * s).  Since every recurrence step multiplies by tc, bins with
tc == 0 never contribute: the (i)DFT only needs the nonzero bins of tc
(~10 of 1025 for the reference init).  Both transforms become tiny matmuls.

Device layout (per core, 4 batch elements), v3 (fp16 pipeline):
  The signal lives in SBUF as fp16 tiles (128 samples-in-chunk, KT, 1024 cols)
  where col = batch*256 + chunk.  Layout changes use the DMA crossbar
  transpose (dma_start_transpose, 16x128 xbar tiles) instead of PE
  transposes, so the PE only runs the DFT matmuls.  All crossbar transposes
  ride ONE queue: concurrent xbar DMAs from two queues clobber adjacent
  destination slices (observed as nondeterministic corruption on HW).
  HBM->SBUF input loads are gpsimd (software DGE) DMAs casting fp32->fp16
  in flight.

  Forward: spec_i needs frame i = [chunk_i, chunk_{i+1}] but
  cos/sin(2*pi*k*(s+1024)/2048) = (-1)^k * cos/sin(2*pi*k*s/2048), so only
  the half-window matrix U is computed; the second half is sign * U shifted
  by one frame.  The forward runs column-group-first (batches 0-1 then 2-3)
  so the recurrence work starts while the PE is still on batches 2-3.  The
  scan state is fp32 internally regardless of operand dtype, so the fp16
  recurrence does not accumulate rounding; tr stays fp32 since a 2^-11
  error in tc compounds over 256 frames.  The inverse needs the stack
  [out_cur; out_prev]: the scan writes rows 0:64 of scat at col k+1, and a
  GpSimd cross-partition copy fills rows 64:128 one column later (the
  leading memset zero provides the overlap-add edge), giving the inverse a
  single 128-row stationary with one uniform column offset.

  The inverse DFT matrices are scaled by 2**10 (compensated in the tanh
  scale) so the hann-window tails stay in fp16 normal range.  When the
  mixer weights are all equal (softmax of the zero mixer), the accumulator
  is plain adds (split DVE/GpSimd) with the weight folded into the store
  cast; accumulator updates run one block late so they never compete with
  the combine/scan chain.
"""

import numpy as np
from contextlib import ExitStack

import concourse.bass as bass
import concourse.tile as tile
from concourse import bacc, mybir
from concourse import bass_utils

F32 = mybir.dt.float32
F16 = mybir.dt.float16
WS = 2048
STEP = 1024
NCOEF = WS // 2 + 1
NBLK = 4
B = 32
T = 262144
NCORES = 8
BL = B // NCORES          # batch per core
NF = T // STEP            # 256 frames/chunks
KT = STEP // 128          # 8 K-tiles of the forward contraction
COLS = BL * NF            # 1024 free columns (batch-major)
MAX_BINS_PER_CHUNK = 32   # 2*nb must fit in a 64-row half
WI_SCALE = 1024.0         # keeps hann tails in fp16 normal range

_CACHE = {}


def _plan_chunks(tc_vec):
    nz = np.nonzero(tc_vec)[0]
    if len(nz) == 0:
        nz = np.array([1], dtype=np.int64)  # dummy bin with tc=0: contributes 0
    chunks = [nz[i:i + MAX_BINS_PER_CHUNK] for i in range(0, len(nz), MAX_BINS_PER_CHUNK)]
    return chunks


def _host_matrices(tc_vec, chunks):
    """Build per-chunk constant arrays (float64 math, fp16/fp32 storage)."""
    hann = 0.5 - 0.5 * np.cos(2.0 * np.pi * np.arange(WS) / WS)
    out = []
    for bins in chunks:
        nb = len(bins)
        k = bins.astype(np.float64)
        tcv = tc_vec[bins].astype(np.float64)
        s = np.arange(STEP, dtype=np.float64)
        ang = 2.0 * np.pi * np.outer(s, k) / WS                      # (1024, nb)
        # duplicated on both column halves: the matmul then writes U to
        # partitions 0:64 and 64:128 at no extra PE cost, keeping both scans
        # partition-aligned
        bf = np.zeros((STEP, 128))
        bf[:, 0:nb] = np.cos(ang) * tcv
        bf[:, nb:2 * nb] = -np.sin(ang) * tcv
        bf[:, 64:64 + 2 * nb] = bf[:, 0:2 * nb]
        bf_t = bf.reshape(KT, 128, 128).transpose(1, 0, 2)           # (128, 8, 128)
        sign = np.zeros((128, 1))
        sign[0:nb, 0] = (-1.0) ** k
        sign[nb:2 * nb, 0] = (-1.0) ** k
        sign[64:64 + 2 * nb] = sign[0:2 * nb]
        tcrep = np.zeros((128, NF))
        tcrep[0:nb] = tcv[:, None]
        tcrep[nb:2 * nb] = tcv[:, None]
        tcrep[64:64 + 2 * nb] = tcrep[0:2 * nb]
        w = np.where((bins == 0) | (bins == WS // 2), 1.0, 2.0)
        s2 = np.arange(WS, dtype=np.float64)
        ang2 = 2.0 * np.pi * np.outer(k, s2) / WS                    # (nb, 2048)
        are = (w[:, None] / WS) * np.cos(ang2) * hann * WI_SCALE
        aim = -(w[:, None] / WS) * np.sin(ang2) * hann * WI_SCALE
        w1 = np.concatenate([are[:, :STEP], aim[:, :STEP]], axis=0)  # cur frame
        w2 = np.concatenate([are[:, STEP:], aim[:, STEP:]], axis=0)  # prev frame
        pad = np.zeros((64 - 2 * nb, WS // 2))
        winv = np.concatenate([w1, pad, w2, pad], axis=0).reshape(128, KT, 128)
        out.append(dict(
            nb=nb,
            bf=np.ascontiguousarray(bf_t, dtype=np.float16),
            winv=np.ascontiguousarray(winv, dtype=np.float16),
            sign=np.ascontiguousarray(sign, dtype=np.float16),
            tcrep=np.ascontiguousarray(tcrep, dtype=np.float32),
        ))
    return out


def _build(chunk_sizes, gains, wmix, debug_taps=False):
    """Trace+compile the Bass program. chunk_sizes: tuple of tuples of nb per block."""
    nc = bacc.Bacc("TRN2", target_bir_lowering=False, debug=False)
    # x arrives host-pre-transposed to the device layout (sample-in-chunk on
    # partitions, k-tile, batch*frame columns) in fp16; the output is the
    # fp16 accumulator in the same layout, un-transposed and scaled on host
    x_d = nc.dram_tensor("x", (128, KT, COLS), F16, kind="ExternalInput").ap()
    out_d = nc.dram_tensor("out", (128, KT, COLS), F16, kind="ExternalOutput").ap()
    taps = {}
    if debug_taps:
        for nm, shp in [("t_x16", (128, KT, COLS)), ("t_dst1", (128, KT, COLS)),
                        ("t_scat1", (128, BL, NF + 1)), ("t_acc", (128, KT, COLS))]:
            taps[nm] = nc.dram_tensor(nm, shp, F16, kind="ExternalOutput").ap()
    # consolidated constants: one (bf, wi) pair for block 1, one blob for the
    # rest, plus all-sign/all-tr blobs — 6 input DMAs instead of 16
    chunks_flat = [(kb, c) for kb in range(NBLK) for c in range(len(chunk_sizes[kb]))]
    nch_tot = len(chunks_flat)
    nch_0 = len(chunk_sizes[0])
    nch_r = nch_tot - nch_0
    cons = {
        "bf0": nc.dram_tensor("bf0", (128, KT, 128 * nch_0), F16, kind="ExternalInput").ap(),
        "wi0": nc.dram_tensor("wi0", (128, KT, 128 * nch_0), F16, kind="ExternalInput").ap(),
        "sgA": nc.dram_tensor("sgA", (128, nch_tot), F16, kind="ExternalInput").ap(),
        "trA": nc.dram_tensor("trA", (128, NF * nch_tot), F32, kind="ExternalInput").ap(),
    }
    if nch_r:
        cons["bfR"] = nc.dram_tensor("bfR", (128, KT, 128 * nch_r), F16, kind="ExternalInput").ap()
        cons["wiR"] = nc.dram_tensor("wiR", (128, KT, 128 * nch_r), F16, kind="ExternalInput").ap()

    mult = mybir.AluOpType.mult
    add = mybir.AluOpType.add
    Tanh = mybir.ActivationFunctionType.Tanh
    Copy = mybir.ActivationFunctionType.Copy

    w_equal = bool(np.allclose(wmix, wmix[0], rtol=1e-7, atol=0.0))

    with tile.TileContext(nc) as tc, ExitStack() as ctx:
        cpool = ctx.enter_context(tc.tile_pool(name="const", bufs=1))
        big = ctx.enter_context(tc.tile_pool(name="big", bufs=1))
        work = ctx.enter_context(tc.tile_pool(name="work", bufs=2))
        # PSUM budget (8 banks): uv (2 banks) x2 + ips (1 bank) x4 = 8
        pmm = ctx.enter_context(tc.tile_pool(name="pmm", bufs=2, space="PSUM"))
        pmi = ctx.enter_context(tc.tile_pool(name="pmi", bufs=4, space="PSUM"))

        # consolidated resident constants; block 1's bf rides first on sync,
        # wi0 on scalar, interleaved with the x k-tile slab DMAs
        bf0_t = cpool.tile([128, KT, 128 * nch_0], F16, tag="bf0", name="bf0_t")
        nc.sync.dma_start(bf0_t[:], cons["bf0"][:])
        wi0_t = cpool.tile([128, KT, 128 * nch_0], F16, tag="wi0", name="wi0_t")
        nc.scalar.dma_start(wi0_t[:], cons["wi0"][:])
        sgA_t = cpool.tile([128, nch_tot], F16, tag="sgA", name="sgA_t")
        trA_t = cpool.tile([128, NF * nch_tot], F32, tag="trA", name="trA_t")
        bfR_t = wiR_t = None
        if nch_r:
            bfR_t = cpool.tile([128, KT, 128 * nch_r], F16, tag="bfR", name="bfR_t")
            wiR_t = cpool.tile([128, KT, 128 * nch_r], F16, tag="wiR", name="wiR_t")

        def bf_ap(kb, c):
            i = chunks_flat.index((kb, c))
            if kb == 0:
                return bf0_t[:, :, 128 * i:128 * (i + 1)]
            return bfR_t[:, :, 128 * (i - nch_0):128 * (i - nch_0 + 1)]

        def wi_ap(kb, c):
            i = chunks_flat.index((kb, c))
            if kb == 0:
                return wi0_t[:, :, 128 * i:128 * (i + 1)]
            return wiR_t[:, :, 128 * (i - nch_0):128 * (i - nch_0 + 1)]

        xbuf = [big.tile([128, KT, COLS], F16, tag=f"xb{i}", name=f"xb{i}") for i in range(2)]
        accb = big.tile([128, KT, COLS], F16, tag="acc", name="acc")
        # scat col k: rows 0:64 = out_{k-1} (scan), rows 64:128 = out_{k-2}
        # (cross-partition shifted copy); col 0 zero feeds the overlap edge
        scat = cpool.tile([128, BL, NF + 1], F16, tag="scat", name="scat")
        nc.vector.memset(scat[:, :, 0:2], 0.0)
        # uvs col NF stays zero: the sign-combine then covers all 256 cols
        uvs = cpool.tile([128, BL, NF + 1], F16, tag="uvs", name="uvs")
        nc.vector.memset(uvs[:, :, NF:NF + 1], 0.0)

        # ---- load x: one DMA per k-tile slab, alternating the two HWDGE
        # rings; the forward's a-th matmul fires as soon as slab a lands ----
        for m in range(KT):
            dma_eng = nc.sync if m % 2 == 0 else nc.scalar
            dma_eng.dma_start(xbuf[0][:, m, :], x_d[:, m, :])
        nc.gpsimd.dma_start(sgA_t[:], cons["sgA"][:])
        nc.sync.dma_start(trA_t[:], cons["trA"][:])
        if nch_r:
            nc.scalar.dma_start(bfR_t[:], cons["bfR"][:])
            nc.scalar.dma_start(wiR_t[:], cons["wiR"][:])
        for m in range(KT):
            if w_equal:
                nc.gpsimd.tensor_copy(accb[:, m, :], xbuf[0][:, m, :])
            else:
                nc.gpsimd.tensor_scalar_mul(accb[:, m, :], xbuf[0][:, m, :],
                                            float(wmix[0]))

        if debug_taps:
            nc.sync.dma_start(taps["t_x16"][:], xbuf[0][:])

        pending_acc = []

        def acc_update(m, t, w, cols, pool=False):
            if w_equal and pool:
                nc.gpsimd.tensor_tensor(accb[:, m, cols], accb[:, m, cols],
                                        t[:, m, cols], op=add)
            elif w_equal:
                nc.vector.tensor_tensor(accb[:, m, cols], accb[:, m, cols],
                                        t[:, m, cols], op=add)
            else:
                nc.vector.scalar_tensor_tensor(
                    accb[:, m, cols], t[:, m, cols], w, accb[:, m, cols], op0=mult, op1=add)

        def flush_acc():
            for m, t, w in pending_acc:
                acc_update(m, t, w, slice(None), pool=(m % 4 == 3))
            pending_acc.clear()

        # ---- block chain ----
        for kb in range(NBLK):
            if debug_taps and kb == 1:
                nc.sync.dma_start(taps["t_dst1"][:], xbuf[1][:])
            src = xbuf[kb % 2]
            dst = xbuf[(kb + 1) % 2]
            sizes = chunk_sizes[kb]
            nch = len(sizes)
            inv_sb = None
            if nch > 1:
                inv_sb = big.tile([128, KT, COLS], F32, tag="is", name=f"is{kb}")
            for c, nb in enumerate(sizes):
                ci = chunks_flat.index((kb, c))
                bf = bf_ap(kb, c)
                wi = wi_ap(kb, c)
                sg = sgA_t[:, ci:ci + 1]
                tr = trA_t[:, NF * ci:NF * (ci + 1)]
                gain = float(gains[kb]) / WI_SCALE

                # two independent column-group streams (batches 0-1 / 2-3):
                # PE runs fwd g0, fwd g1, inv g0, inv g1 back to back; the
                # recurrence for each group overlaps the PE work of the other,
                # and the next block's fwd g0 only waits on this block's g0
                # tanh halves
                def combine(b):
                    nc.scalar.copy(uvs[:, b, 0:NF], uv[b // 2][:, b % 2, :])
                    in1 = work.tile([128, NF], F16, tag="in1", name=f"in1_{kb}_{c}_{b}")
                    nc.vector.scalar_tensor_tensor(
                        in1[:], uvs[:, b, 1:NF + 1], sg,
                        uvs[:, b, 0:NF], op0=mult, op1=add)
                    nc.vector.tensor_tensor_scan(
                        scat[0:64, b, 1:NF + 1], tr[0:64, :], in1[0:64, :],
                        initial=0.0, op0=mult, op1=add)
                    nc.gpsimd.tensor_copy(scat[64:128, b, 1:NF + 1],
                                          scat[0:64, b, 0:NF])

                uv = [None, None]
                for g in range(2):
                    uv[g] = pmm.tile([128, 2, NF], F32, tag="uv", name=f"uv{kb}_{c}_{g}")
                    for a in range(KT):
                        nc.tensor.matmul(uv[g][:], bf[:, a, :],
                                         src[:, a, g * 512:(g + 1) * 512],
                                         start=(a == 0), stop=(a == KT - 1))
                    combine(2 * g)
                    combine(2 * g + 1)
                if debug_taps and kb == 0:
                    nc.scalar.dma_start(taps["t_scat1"][:], scat[:])
                flush_acc()
                last = kb == NBLK - 1 and c == nch - 1

                def inv_mm(g, m):
                    ip = pmi.tile([128, 512], F32, tag="ips", name=f"ip{kb}_{c}_{g}_{m}")
                    nc.tensor.matmul(ip[:], wi[:, m, :],
                                     scat[:, 2 * g:2 * g + 2, 1:NF + 1],
                                     start=True, stop=True)
                    if nch == 1:
                        return ip, dst[:, m, g * 512:(g + 1) * 512]
                    half = inv_sb[:, m, g * 512:(g + 1) * 512]
                    if c == 0:
                        nc.vector.tensor_copy(half, ip[:])
                    else:
                        nc.vector.tensor_add(half, half, ip[:])
                    return ip, half

                for m in range(KT):
                    ip0, h0 = inv_mm(0, m)
                    if nch == 1:
                        nc.scalar.activation(h0, ip0[:], Tanh, scale=gain)
                for m in range(KT):
                    ip1, h1 = inv_mm(1, m)
                    if nch == 1:
                        nc.scalar.activation(h1, ip1[:], Tanh, scale=gain)
                    elif c == nch - 1:
                        nc.scalar.activation(dst[:, m, :], inv_sb[:, m, :], Tanh,
                                             scale=gain)
                    if c == nch - 1:
                        if last:
                            # final block: accumulate and DMA this m-tile out
                            # right away so the store streams while the
                            # remaining inverse columns still compute
                            acc_update(m, dst, float(wmix[kb + 1]), slice(None))
                            dma_eng = nc.sync if m % 2 == 0 else nc.scalar
                            dma_eng.dma_start(out_d[:, m, :], accb[:, m, :])
                        else:
                            pending_acc.append((m, dst, float(wmix[kb + 1])))

        # stores already streamed out inside block 4
        pending_acc.clear()
        if debug_taps:
            nc.sync.dma_start(taps["t_acc"][:], accb[:])

    nc.compile()
    return nc


def _const_map(transfers, plans):
    bfs, wis, sgs, trs, nch0 = [], [], [], [], None
    for kb in range(NBLK):
        mats = _host_matrices(transfers[kb].astype(np.float64), plans[kb])
        if kb == 0:
            nch0 = len(mats)
        for md in mats:
            bfs.append(md["bf"])
            wis.append(md["winv"])
            sgs.append(md["sign"])
            trs.append(md["tcrep"])
    cm = {
        "bf0": np.ascontiguousarray(np.concatenate(bfs[:nch0], axis=2)),
        "wi0": np.ascontiguousarray(np.concatenate(wis[:nch0], axis=2)),
        "sgA": np.ascontiguousarray(np.concatenate(sgs, axis=1)),
        "trA": np.ascontiguousarray(np.concatenate(trs, axis=1)),
    }
    if len(bfs) > nch0:
        cm["bfR"] = np.ascontiguousarray(np.concatenate(bfs[nch0:], axis=2))
        cm["wiR"] = np.ascontiguousarray(np.concatenate(wis[nch0:], axis=2))
    return cm


def _in_maps(x, const_map):
    # host-side shard + transpose to the device layout:
    # x16t[p, m, b*NF + f] = x[b, f*1024 + m*128 + p]
    xr = np.asarray(x, dtype=np.float32).reshape(B, T).astype(np.float16)
    maps = []
    for core in range(NCORES):
        xc = xr[core * BL:(core + 1) * BL].reshape(BL, NF, KT, 128)
        m = dict(const_map)
        m["x"] = np.ascontiguousarray(xc.transpose(3, 2, 0, 1).reshape(128, KT, COLS))
        maps.append(m)
    return maps


def _out_gather(res, scale):
    outs = []
    for i in range(NCORES):
        ot = res.results[i]["out"].reshape(128, KT, BL, NF)
        oc = ot.transpose(2, 3, 1, 0).reshape(BL, 1, T)
        outs.append(oc)
    return (np.concatenate(outs, axis=0).astype(np.float32) * np.float32(scale))


def kernel(x, transfers, gains, mixer):
    transfers = np.asarray(transfers, dtype=np.float32)
    gains = np.asarray(gains, dtype=np.float64)
    mixer = np.asarray(mixer, dtype=np.float64)
    wm = np.exp(mixer - mixer.max())
    wm = wm / wm.sum()

    plans = [_plan_chunks(transfers[kb]) for kb in range(NBLK)]
    chunk_sizes = tuple(tuple(len(ch) for ch in pl) for pl in plans)
    key = (chunk_sizes, tuple(np.round(gains, 9)), tuple(np.round(wm, 9)))
    if key not in _CACHE:
        _CACHE[key] = _build(chunk_sizes, gains, wm)
    nc = _CACHE[key]

    in_maps = _in_maps(x, _const_map(transfers, plans))
    res = bass_utils.run_bass_kernel_spmd(nc, in_maps, core_ids=list(range(NCORES)))
    w_equal = bool(np.allclose(wm, wm[0], rtol=1e-7, atol=0.0))
    return _out_gather(res, wm[0] if w_equal else 1.0)


# revision 78
# speedup vs baseline: 1.2226x; 1.2049x over previous
"""Trainium2 Bass kernel for nn_AudioNetwork (4-block STFT resonator chain).

Algorithm notes
---------------
Per block: frame x (win 2048, hop 1024), rfft, per-bin linear recurrence over
frames out_i = (spec_i + out_{i-1}) * tc, irfft, hann-windowed overlap-add,
tanh(gain * s).  Since every recurrence step multiplies by tc, bins with
tc == 0 never contribute: the (i)DFT only needs the nonzero bins of tc
(~10 of 1025 for the reference init).  Both transforms become tiny matmuls.

Device layout (per core, 4 batch elements), v3 (fp16 pipeline):
  The signal lives in SBUF as fp16 tiles (128 samples-in-chunk, KT, 1024 cols)
  where col = batch*256 + chunk.  Layout changes use the DMA crossbar
  transpose (dma_start_transpose, 16x128 xbar tiles) instead of PE
  transposes, so the PE only runs the DFT matmuls.  All crossbar transposes
  ride ONE queue: concurrent xbar DMAs from two queues clobber adjacent
  destination slices (observed as nondeterministic corruption on HW).
  HBM->SBUF input loads are gpsimd (software DGE) DMAs casting fp32->fp16
  in flight.

  Forward: spec_i needs frame i = [chunk_i, chunk_{i+1}] but
  cos/sin(2*pi*k*(s+1024)/2048) = (-1)^k * cos/sin(2*pi*k*s/2048), so only
  the half-window matrix U is computed; the second half is sign * U shifted
  by one frame.  The forward runs column-group-first (batches 0-1 then 2-3)
  so the recurrence work starts while the PE is still on batches 2-3.  The
  scan state is fp32 internally regardless of operand dtype, so the fp16
  recurrence does not accumulate rounding; tr stays fp32 since a 2^-11
  error in tc compounds over 256 frames.  The inverse needs the stack
  [out_cur; out_prev]: the scan writes rows 0:64 of scat at col k+1, and a
  GpSimd cross-partition copy fills rows 64:128 one column later (the
  leading memset zero provides the overlap-add edge), giving the inverse a
  single 128-row stationary with one uniform column offset.

  The inverse DFT matrices are scaled by 2**10 (compensated in the tanh
  scale) so the hann-window tails stay in fp16 normal range.  When the
  mixer weights are all equal (softmax of the zero mixer), the accumulator
  is plain adds (split DVE/GpSimd) with the weight folded into the store
  cast; accumulator updates run one block late so they never compete with
  the combine/scan chain.
"""

import numpy as np
from contextlib import ExitStack

import concourse.bass as bass
import concourse.tile as tile
from concourse import bacc, mybir
from concourse import bass_utils

F32 = mybir.dt.float32
F16 = mybir.dt.float16
WS = 2048
STEP = 1024
NCOEF = WS // 2 + 1
NBLK = 4
B = 32
T = 262144
NCORES = 8
BL = B // NCORES          # batch per core
NF = T // STEP            # 256 frames/chunks
KT = STEP // 128          # 8 K-tiles of the forward contraction
COLS = BL * NF            # 1024 free columns (batch-major)
MAX_BINS_PER_CHUNK = 32   # 2*nb must fit in a 64-row half
WI_SCALE = 1024.0         # keeps hann tails in fp16 normal range

_CACHE = {}


def _plan_chunks(tc_vec):
    nz = np.nonzero(tc_vec)[0]
    if len(nz) == 0:
        nz = np.array([1], dtype=np.int64)  # dummy bin with tc=0: contributes 0
    chunks = [nz[i:i + MAX_BINS_PER_CHUNK] for i in range(0, len(nz), MAX_BINS_PER_CHUNK)]
    return chunks


def _host_matrices(tc_vec, chunks):
    """Build per-chunk constant arrays (float64 math, fp16/fp32 storage)."""
    hann = 0.5 - 0.5 * np.cos(2.0 * np.pi * np.arange(WS) / WS)
    out = []
    for bins in chunks:
        nb = len(bins)
        k = bins.astype(np.float64)
        tcv = tc_vec[bins].astype(np.float64)
        s = np.arange(STEP, dtype=np.float64)
        ang = 2.0 * np.pi * np.outer(s, k) / WS                      # (1024, nb)
        # duplicated on both column halves: the matmul then writes U to
        # partitions 0:64 and 64:128 at no extra PE cost, keeping both scans
        # partition-aligned
        bf = np.zeros((STEP, 128))
        bf[:, 0:nb] = np.cos(ang) * tcv
        bf[:, nb:2 * nb] = -np.sin(ang) * tcv
        bf[:, 64:64 + 2 * nb] = bf[:, 0:2 * nb]
        bf_t = bf.reshape(KT, 128, 128).transpose(1, 0, 2)           # (128, 8, 128)
        sign = np.zeros((128, 1))
        sign[0:nb, 0] = (-1.0) ** k
        sign[nb:2 * nb, 0] = (-1.0) ** k
        sign[64:64 + 2 * nb] = sign[0:2 * nb]
        tcrep = np.zeros((128, NF))
        tcrep[0:nb] = tcv[:, None]
        tcrep[nb:2 * nb] = tcv[:, None]
        tcrep[64:64 + 2 * nb] = tcrep[0:2 * nb]
        w = np.where((bins == 0) | (bins == WS // 2), 1.0, 2.0)
        s2 = np.arange(WS, dtype=np.float64)
        ang2 = 2.0 * np.pi * np.outer(k, s2) / WS                    # (nb, 2048)
        are = (w[:, None] / WS) * np.cos(ang2) * hann * WI_SCALE
        aim = -(w[:, None] / WS) * np.sin(ang2) * hann * WI_SCALE
        w1 = np.concatenate([are[:, :STEP], aim[:, :STEP]], axis=0)  # cur frame
        w2 = np.concatenate([are[:, STEP:], aim[:, STEP:]], axis=0)  # prev frame
        pad = np.zeros((64 - 2 * nb, WS // 2))
        winv = np.concatenate([w1, pad, w2, pad], axis=0).reshape(128, KT, 128)
        out.append(dict(
            nb=nb,
            bf=np.ascontiguousarray(bf_t, dtype=np.float16),
            winv=np.ascontiguousarray(winv, dtype=np.float16),
            sign=np.ascontiguousarray(sign, dtype=np.float16),
            tcrep=np.ascontiguousarray(tcrep, dtype=np.float32),
        ))
    return out


def _build(chunk_sizes, gains, wmix, debug_taps=False):
    """Trace+compile the Bass program. chunk_sizes: tuple of tuples of nb per block."""
    nc = bacc.Bacc("TRN2", target_bir_lowering=False, debug=False)
    # x arrives host-pre-transposed to the device layout (sample-in-chunk on
    # partitions, k-tile, batch*frame columns) in fp16; the output is the
    # fp16 accumulator in the same layout, un-transposed and scaled on host
    x_d = nc.dram_tensor("x", (128, KT, COLS), F16, kind="ExternalInput").ap()
    out_d = nc.dram_tensor("out", (128, KT, COLS), F16, kind="ExternalOutput").ap()
    taps = {}
    if debug_taps:
        for nm, shp in [("t_x16", (128, KT, COLS)), ("t_dst1", (128, KT, COLS)),
                        ("t_scat1", (128, BL, NF + 1)), ("t_acc", (128, KT, COLS))]:
            taps[nm] = nc.dram_tensor(nm, shp, F16, kind="ExternalOutput").ap()
    # consolidated constants: one (bf, wi) pair for block 1, one blob for the
    # rest, plus all-sign/all-tr blobs — 6 input DMAs instead of 16
    chunks_flat = [(kb, c) for kb in range(NBLK) for c in range(len(chunk_sizes[kb]))]
    nch_tot = len(chunks_flat)
    nch_0 = len(chunk_sizes[0])
    nch_r = nch_tot - nch_0
    cons = {
        "bf0": nc.dram_tensor("bf0", (128, KT, 128 * nch_0), F16, kind="ExternalInput").ap(),
        "wi0": nc.dram_tensor("wi0", (128, KT, 128 * nch_0), F16, kind="ExternalInput").ap(),
        "sgA": nc.dram_tensor("sgA", (128, nch_tot), F16, kind="ExternalInput").ap(),
        "trA": nc.dram_tensor("trA", (128, NF * nch_tot), F32, kind="ExternalInput").ap(),
    }
    if nch_r:
        cons["bfR"] = nc.dram_tensor("bfR", (128, KT, 128 * nch_r), F16, kind="ExternalInput").ap()
        cons["wiR"] = nc.dram_tensor("wiR", (128, KT, 128 * nch_r), F16, kind="ExternalInput").ap()

    mult = mybir.AluOpType.mult
    add = mybir.AluOpType.add
    Tanh = mybir.ActivationFunctionType.Tanh
    Copy = mybir.ActivationFunctionType.Copy

    w_equal = bool(np.allclose(wmix, wmix[0], rtol=1e-7, atol=0.0))

    with tile.TileContext(nc) as tc, ExitStack() as ctx:
        cpool = ctx.enter_context(tc.tile_pool(name="const", bufs=1))
        big = ctx.enter_context(tc.tile_pool(name="big", bufs=1))
        work = ctx.enter_context(tc.tile_pool(name="work", bufs=2))
        # PSUM budget (8 banks): uv (2 banks) x2 + ips (1 bank) x4 = 8
        pmm = ctx.enter_context(tc.tile_pool(name="pmm", bufs=2, space="PSUM"))
        pmi = ctx.enter_context(tc.tile_pool(name="pmi", bufs=4, space="PSUM"))

        # consolidated resident constants; block 1's bf rides first on sync,
        # wi0 on scalar, interleaved with the x k-tile slab DMAs
        bf0_t = cpool.tile([128, KT, 128 * nch_0], F16, tag="bf0", name="bf0_t")
        nc.sync.dma_start(bf0_t[:], cons["bf0"][:])
        wi0_t = cpool.tile([128, KT, 128 * nch_0], F16, tag="wi0", name="wi0_t")
        nc.scalar.dma_start(wi0_t[:], cons["wi0"][:])
        sgA_t = cpool.tile([128, nch_tot], F16, tag="sgA", name="sgA_t")
        trA_t = cpool.tile([128, NF * nch_tot], F32, tag="trA", name="trA_t")
        bfR_t = wiR_t = None
        if nch_r:
            bfR_t = cpool.tile([128, KT, 128 * nch_r], F16, tag="bfR", name="bfR_t")
            wiR_t = cpool.tile([128, KT, 128 * nch_r], F16, tag="wiR", name="wiR_t")

        def bf_ap(kb, c):
            i = chunks_flat.index((kb, c))
            if kb == 0:
                return bf0_t[:, :, 128 * i:128 * (i + 1)]
            return bfR_t[:, :, 128 * (i - nch_0):128 * (i - nch_0 + 1)]

        def wi_ap(kb, c):
            i = chunks_flat.index((kb, c))
            if kb == 0:
                return wi0_t[:, :, 128 * i:128 * (i + 1)]
            return wiR_t[:, :, 128 * (i - nch_0):128 * (i - nch_0 + 1)]

        xbuf = [big.tile([128, KT, COLS], F16, tag=f"xb{i}", name=f"xb{i}") for i in range(2)]
        accb = big.tile([128, KT, COLS], F16, tag="acc", name="acc")
        # scat col k: rows 0:64 = out_{k-1} (scan), rows 64:128 = out_{k-2}
        # (cross-partition shifted copy); col 0 zero feeds the overlap edge
        scat = cpool.tile([128, BL, NF + 1], F16, tag="scat", name="scat")
        nc.vector.memset(scat[:, :, 0:2], 0.0)
        # uvs col NF stays zero: the sign-combine then covers all 256 cols
        uvs = cpool.tile([128, BL, NF + 1], F16, tag="uvs", name="uvs")
        nc.vector.memset(uvs[:, :, NF:NF + 1], 0.0)

        # ---- load x: one DMA per k-tile slab, alternating the two HWDGE
        # rings; the forward's a-th matmul fires as soon as slab a lands ----
        for m in range(KT):
            dma_eng = nc.sync if m % 2 == 0 else nc.scalar
            dma_eng.dma_start(xbuf[0][:, m, :], x_d[:, m, :])
        nc.gpsimd.dma_start(sgA_t[:], cons["sgA"][:])
        nc.sync.dma_start(trA_t[:], cons["trA"][:])
        if nch_r:
            nc.scalar.dma_start(bfR_t[:], cons["bfR"][:])
            nc.scalar.dma_start(wiR_t[:], cons["wiR"][:])
        for m in range(KT):
            if w_equal:
                nc.vector.tensor_copy(accb[:, m, :], xbuf[0][:, m, :])
            else:
                nc.vector.tensor_scalar_mul(accb[:, m, :], xbuf[0][:, m, :],
                                            float(wmix[0]))

        if debug_taps:
            nc.sync.dma_start(taps["t_x16"][:], xbuf[0][:])

        pending_acc = []

        def acc_update(m, t, w, cols, pool=False):
            if w_equal and pool:
                nc.gpsimd.tensor_tensor(accb[:, m, cols], accb[:, m, cols],
                                        t[:, m, cols], op=add)
            elif w_equal:
                nc.vector.tensor_tensor(accb[:, m, cols], accb[:, m, cols],
                                        t[:, m, cols], op=add)
            else:
                nc.vector.scalar_tensor_tensor(
                    accb[:, m, cols], t[:, m, cols], w, accb[:, m, cols], op0=mult, op1=add)

        def flush_acc():
            for m, t, w in pending_acc:
                acc_update(m, t, w, slice(None), pool=(m % 4 == 3))
            pending_acc.clear()

        # ---- block chain ----
        for kb in range(NBLK):
            if debug_taps and kb == 1:
                nc.sync.dma_start(taps["t_dst1"][:], xbuf[1][:])
            src = xbuf[kb % 2]
            dst = xbuf[(kb + 1) % 2]
            sizes = chunk_sizes[kb]
            nch = len(sizes)
            inv_sb = None
            if nch > 1:
                inv_sb = big.tile([128, KT, COLS], F32, tag="is", name=f"is{kb}")
            for c, nb in enumerate(sizes):
                ci = chunks_flat.index((kb, c))
                bf = bf_ap(kb, c)
                wi = wi_ap(kb, c)
                sg = sgA_t[:, ci:ci + 1]
                tr = trA_t[:, NF * ci:NF * (ci + 1)]
                gain = float(gains[kb]) / WI_SCALE

                # two independent column-group streams (batches 0-1 / 2-3):
                # PE runs fwd g0, fwd g1, inv g0, inv g1 back to back; the
                # recurrence for each group overlaps the PE work of the other,
                # and the next block's fwd g0 only waits on this block's g0
                # tanh halves
                def combine(b):
                    nc.scalar.copy(uvs[:, b, 0:NF], uv[b // 2][:, b % 2, :])
                    in1 = work.tile([128, NF], F16, tag="in1", name=f"in1_{kb}_{c}_{b}")
                    nc.vector.scalar_tensor_tensor(
                        in1[:], uvs[:, b, 1:NF + 1], sg,
                        uvs[:, b, 0:NF], op0=mult, op1=add)
                    nc.vector.tensor_tensor_scan(
                        scat[0:64, b, 1:NF + 1], tr[0:64, :], in1[0:64, :],
                        initial=0.0, op0=mult, op1=add)
                    nc.gpsimd.tensor_copy(scat[64:128, b, 1:NF + 1],
                                          scat[0:64, b, 0:NF])

                uv = [None, None]
                for g in range(2):
                    uv[g] = pmm.tile([128, 2, NF], F32, tag="uv", name=f"uv{kb}_{c}_{g}")
                    for a in range(KT):
                        nc.tensor.matmul(uv[g][:], bf[:, a, :],
                                         src[:, a, g * 512:(g + 1) * 512],
                                         start=(a == 0), stop=(a == KT - 1))
                    combine(2 * g)
                    combine(2 * g + 1)
                if debug_taps and kb == 0:
                    nc.scalar.dma_start(taps["t_scat1"][:], scat[:])
                flush_acc()
                last = kb == NBLK - 1 and c == nch - 1

                def inv_mm(g, m):
                    ip = pmi.tile([128, 512], F32, tag="ips", name=f"ip{kb}_{c}_{g}_{m}")
                    nc.tensor.matmul(ip[:], wi[:, m, :],
                                     scat[:, 2 * g:2 * g + 2, 1:NF + 1],
                                     start=True, stop=True)
                    if nch == 1:
                        return ip, dst[:, m, g * 512:(g + 1) * 512]
                    half = inv_sb[:, m, g * 512:(g + 1) * 512]
                    if c == 0:
                        nc.vector.tensor_copy(half, ip[:])
                    else:
                        nc.vector.tensor_add(half, half, ip[:])
                    return ip, half

                for m in range(KT):
                    ip0, h0 = inv_mm(0, m)
                    if nch == 1:
                        nc.scalar.activation(h0, ip0[:], Tanh, scale=gain)
                for m in range(KT):
                    ip1, h1 = inv_mm(1, m)
                    if nch == 1:
                        nc.scalar.activation(h1, ip1[:], Tanh, scale=gain)
                    elif c == nch - 1:
                        nc.scalar.activation(dst[:, m, :], inv_sb[:, m, :], Tanh,
                                             scale=gain)
                    if c == nch - 1:
                        if last:
                            # final block: accumulate and DMA this m-tile out
                            # right away so the store streams while the
                            # remaining inverse columns still compute
                            acc_update(m, dst, float(wmix[kb + 1]), slice(None))
                            dma_eng = nc.sync if m % 2 == 0 else nc.scalar
                            dma_eng.dma_start(out_d[:, m, :], accb[:, m, :])
                        else:
                            pending_acc.append((m, dst, float(wmix[kb + 1])))

        # stores already streamed out inside block 4
        pending_acc.clear()
        if debug_taps:
            nc.sync.dma_start(taps["t_acc"][:], accb[:])

    nc.compile()
    return nc


def _const_map(transfers, plans):
    bfs, wis, sgs, trs, nch0 = [], [], [], [], None
    for kb in range(NBLK):
        mats = _host_matrices(transfers[kb].astype(np.float64), plans[kb])
        if kb == 0:
            nch0 = len(mats)
        for md in mats:
            bfs.append(md["bf"])
            wis.append(md["winv"])
            sgs.append(md["sign"])
            trs.append(md["tcrep"])
    cm = {
        "bf0": np.ascontiguousarray(np.concatenate(bfs[:nch0], axis=2)),
        "wi0": np.ascontiguousarray(np.concatenate(wis[:nch0], axis=2)),
        "sgA": np.ascontiguousarray(np.concatenate(sgs, axis=1)),
        "trA": np.ascontiguousarray(np.concatenate(trs, axis=1)),
    }
    if len(bfs) > nch0:
        cm["bfR"] = np.ascontiguousarray(np.concatenate(bfs[nch0:], axis=2))
        cm["wiR"] = np.ascontiguousarray(np.concatenate(wis[nch0:], axis=2))
    return cm


def _in_maps(x, const_map):
    # host-side shard + transpose to the device layout:
    # x16t[p, m, b*NF + f] = x[b, f*1024 + m*128 + p]
    xr = np.asarray(x, dtype=np.float32).reshape(B, T).astype(np.float16)
    maps = []
    for core in range(NCORES):
        xc = xr[core * BL:(core + 1) * BL].reshape(BL, NF, KT, 128)
        m = dict(const_map)
        m["x"] = np.ascontiguousarray(xc.transpose(3, 2, 0, 1).reshape(128, KT, COLS))
        maps.append(m)
    return maps


def _out_gather(res, scale):
    outs = []
    for i in range(NCORES):
        ot = res.results[i]["out"].reshape(128, KT, BL, NF)
        oc = ot.transpose(2, 3, 1, 0).reshape(BL, 1, T)
        outs.append(oc)
    return (np.concatenate(outs, axis=0).astype(np.float32) * np.float32(scale))


def kernel(x, transfers, gains, mixer):
    transfers = np.asarray(transfers, dtype=np.float32)
    gains = np.asarray(gains, dtype=np.float64)
    mixer = np.asarray(mixer, dtype=np.float64)
    wm = np.exp(mixer - mixer.max())
    wm = wm / wm.sum()

    plans = [_plan_chunks(transfers[kb]) for kb in range(NBLK)]
    chunk_sizes = tuple(tuple(len(ch) for ch in pl) for pl in plans)
    key = (chunk_sizes, tuple(np.round(gains, 9)), tuple(np.round(wm, 9)))
    if key not in _CACHE:
        _CACHE[key] = _build(chunk_sizes, gains, wm)
    nc = _CACHE[key]

    in_maps = _in_maps(x, _const_map(transfers, plans))
    res = bass_utils.run_bass_kernel_spmd(nc, in_maps, core_ids=list(range(NCORES)))
    w_equal = bool(np.allclose(wm, wm[0], rtol=1e-7, atol=0.0))
    return _out_gather(res, wm[0] if w_equal else 1.0)


# revision 79
# speedup vs baseline: 1.2484x; 1.0212x over previous
"""Trainium2 Bass kernel for nn_AudioNetwork (4-block STFT resonator chain).

Algorithm notes
---------------
Per block: frame x (win 2048, hop 1024), rfft, per-bin linear recurrence over
frames out_i = (spec_i + out_{i-1}) * tc, irfft, hann-windowed overlap-add,
tanh(gain * s).  Since every recurrence step multiplies by tc, bins with
tc == 0 never contribute: the (i)DFT only needs the nonzero bins of tc
(~10 of 1025 for the reference init).  Both transforms become tiny matmuls.

Device layout (per core, 4 batch elements), v3 (fp16 pipeline):
  The signal lives in SBUF as fp16 tiles (128 samples-in-chunk, KT, 1024 cols)
  where col = batch*256 + chunk.  Layout changes use the DMA crossbar
  transpose (dma_start_transpose, 16x128 xbar tiles) instead of PE
  transposes, so the PE only runs the DFT matmuls.  All crossbar transposes
  ride ONE queue: concurrent xbar DMAs from two queues clobber adjacent
  destination slices (observed as nondeterministic corruption on HW).
  HBM->SBUF input loads are gpsimd (software DGE) DMAs casting fp32->fp16
  in flight.

  Forward: spec_i needs frame i = [chunk_i, chunk_{i+1}] but
  cos/sin(2*pi*k*(s+1024)/2048) = (-1)^k * cos/sin(2*pi*k*s/2048), so only
  the half-window matrix U is computed; the second half is sign * U shifted
  by one frame.  The forward runs column-group-first (batches 0-1 then 2-3)
  so the recurrence work starts while the PE is still on batches 2-3.  The
  scan state is fp32 internally regardless of operand dtype, so the fp16
  recurrence does not accumulate rounding; tr stays fp32 since a 2^-11
  error in tc compounds over 256 frames.  The inverse needs the stack
  [out_cur; out_prev]: the scan writes rows 0:64 of scat at col k+1, and a
  GpSimd cross-partition copy fills rows 64:128 one column later (the
  leading memset zero provides the overlap-add edge), giving the inverse a
  single 128-row stationary with one uniform column offset.

  The inverse DFT matrices are scaled by 2**10 (compensated in the tanh
  scale) so the hann-window tails stay in fp16 normal range.  When the
  mixer weights are all equal (softmax of the zero mixer), the accumulator
  is plain adds (split DVE/GpSimd) with the weight folded into the store
  cast; accumulator updates run one block late so they never compete with
  the combine/scan chain.
"""

import numpy as np
from contextlib import ExitStack

import concourse.bass as bass
import concourse.tile as tile
from concourse import bacc, mybir
from concourse import bass_utils

F32 = mybir.dt.float32
F16 = mybir.dt.float16
WS = 2048
STEP = 1024
NCOEF = WS // 2 + 1
NBLK = 4
B = 32
T = 262144
NCORES = 8
BL = B // NCORES          # batch per core
NF = T // STEP            # 256 frames/chunks
KT = STEP // 128          # 8 K-tiles of the forward contraction
COLS = BL * NF            # 1024 free columns (batch-major)
MAX_BINS_PER_CHUNK = 32   # 2*nb must fit in a 64-row half
WI_SCALE = 1024.0         # keeps hann tails in fp16 normal range

_CACHE = {}


def _plan_chunks(tc_vec):
    nz = np.nonzero(tc_vec)[0]
    if len(nz) == 0:
        nz = np.array([1], dtype=np.int64)  # dummy bin with tc=0: contributes 0
    chunks = [nz[i:i + MAX_BINS_PER_CHUNK] for i in range(0, len(nz), MAX_BINS_PER_CHUNK)]
    return chunks


def _host_matrices(tc_vec, chunks):
    """Build per-chunk constant arrays (float64 math, fp16/fp32 storage)."""
    hann = 0.5 - 0.5 * np.cos(2.0 * np.pi * np.arange(WS) / WS)
    out = []
    for bins in chunks:
        nb = len(bins)
        k = bins.astype(np.float64)
        tcv = tc_vec[bins].astype(np.float64)
        s = np.arange(STEP, dtype=np.float64)
        ang = 2.0 * np.pi * np.outer(s, k) / WS                      # (1024, nb)
        # duplicated on both column halves: the matmul then writes U to
        # partitions 0:64 and 64:128 at no extra PE cost, keeping both scans
        # partition-aligned
        bf = np.zeros((STEP, 128))
        bf[:, 0:nb] = np.cos(ang) * tcv
        bf[:, nb:2 * nb] = -np.sin(ang) * tcv
        bf[:, 64:64 + 2 * nb] = bf[:, 0:2 * nb]
        bf_t = bf.reshape(KT, 128, 128).transpose(1, 0, 2)           # (128, 8, 128)
        sign = np.zeros((128, 1))
        sign[0:nb, 0] = (-1.0) ** k
        sign[nb:2 * nb, 0] = (-1.0) ** k
        sign[64:64 + 2 * nb] = sign[0:2 * nb]
        tcrep = np.zeros((128, NF))
        tcrep[0:nb] = tcv[:, None]
        tcrep[nb:2 * nb] = tcv[:, None]
        tcrep[64:64 + 2 * nb] = tcrep[0:2 * nb]
        w = np.where((bins == 0) | (bins == WS // 2), 1.0, 2.0)
        s2 = np.arange(WS, dtype=np.float64)
        ang2 = 2.0 * np.pi * np.outer(k, s2) / WS                    # (nb, 2048)
        are = (w[:, None] / WS) * np.cos(ang2) * hann * WI_SCALE
        aim = -(w[:, None] / WS) * np.sin(ang2) * hann * WI_SCALE
        w1 = np.concatenate([are[:, :STEP], aim[:, :STEP]], axis=0)  # cur frame
        w2 = np.concatenate([are[:, STEP:], aim[:, STEP:]], axis=0)  # prev frame
        pad = np.zeros((64 - 2 * nb, WS // 2))
        winv = np.concatenate([w1, pad, w2, pad], axis=0).reshape(128, KT, 128)
        out.append(dict(
            nb=nb,
            bf=np.ascontiguousarray(bf_t, dtype=np.float16),
            winv=np.ascontiguousarray(winv, dtype=np.float16),
            sign=np.ascontiguousarray(sign, dtype=np.float16),
            tcrep=np.ascontiguousarray(tcrep, dtype=np.float32),
        ))
    return out


def _build(chunk_sizes, gains, wmix, debug_taps=False):
    """Trace+compile the Bass program. chunk_sizes: tuple of tuples of nb per block."""
    nc = bacc.Bacc("TRN2", target_bir_lowering=False, debug=False)
    # x arrives host-pre-transposed to the device layout (sample-in-chunk on
    # partitions, k-tile, batch*frame columns) in fp16; the output is the
    # fp16 accumulator in the same layout, un-transposed and scaled on host
    x_d = nc.dram_tensor("x", (128, KT, COLS), F16, kind="ExternalInput").ap()
    out_d = nc.dram_tensor("out", (128, KT, COLS), F16, kind="ExternalOutput").ap()
    taps = {}
    if debug_taps:
        for nm, shp in [("t_x16", (128, KT, COLS)), ("t_dst1", (128, KT, COLS)),
                        ("t_scat1", (128, BL, NF + 1)), ("t_acc", (128, KT, COLS))]:
            taps[nm] = nc.dram_tensor(nm, shp, F16, kind="ExternalOutput").ap()
    # consolidated constants: one (bf, wi) pair for block 1, one blob for the
    # rest, plus all-sign/all-tr blobs — 6 input DMAs instead of 16
    chunks_flat = [(kb, c) for kb in range(NBLK) for c in range(len(chunk_sizes[kb]))]
    nch_tot = len(chunks_flat)
    nch_0 = len(chunk_sizes[0])
    nch_r = nch_tot - nch_0
    cons = {
        "bf0": nc.dram_tensor("bf0", (128, KT, 128 * nch_0), F16, kind="ExternalInput").ap(),
        "wi0": nc.dram_tensor("wi0", (128, KT, 128 * nch_0), F16, kind="ExternalInput").ap(),
        "sgA": nc.dram_tensor("sgA", (128, nch_tot), F16, kind="ExternalInput").ap(),
        "trA": nc.dram_tensor("trA", (128, NF * nch_tot), F32, kind="ExternalInput").ap(),
    }
    if nch_r:
        cons["bfR"] = nc.dram_tensor("bfR", (128, KT, 128 * nch_r), F16, kind="ExternalInput").ap()
        cons["wiR"] = nc.dram_tensor("wiR", (128, KT, 128 * nch_r), F16, kind="ExternalInput").ap()

    mult = mybir.AluOpType.mult
    add = mybir.AluOpType.add
    Tanh = mybir.ActivationFunctionType.Tanh
    Copy = mybir.ActivationFunctionType.Copy

    w_equal = bool(np.allclose(wmix, wmix[0], rtol=1e-7, atol=0.0))

    with tile.TileContext(nc) as tc, ExitStack() as ctx:
        cpool = ctx.enter_context(tc.tile_pool(name="const", bufs=1))
        big = ctx.enter_context(tc.tile_pool(name="big", bufs=1))
        work = ctx.enter_context(tc.tile_pool(name="work", bufs=2))
        # PSUM budget (8 banks): uv (2 banks) x2 + ips (1 bank) x4 = 8
        pmm = ctx.enter_context(tc.tile_pool(name="pmm", bufs=2, space="PSUM"))
        pmi = ctx.enter_context(tc.tile_pool(name="pmi", bufs=4, space="PSUM"))

        # consolidated resident constants; block 1's bf rides first on sync,
        # wi0 on scalar, interleaved with the x k-tile slab DMAs
        bf0_t = cpool.tile([128, KT, 128 * nch_0], F16, tag="bf0", name="bf0_t")
        nc.sync.dma_start(bf0_t[:], cons["bf0"][:])
        wi0_t = cpool.tile([128, KT, 128 * nch_0], F16, tag="wi0", name="wi0_t")
        nc.scalar.dma_start(wi0_t[:], cons["wi0"][:])
        sgA_t = cpool.tile([128, nch_tot], F16, tag="sgA", name="sgA_t")
        trA_t = cpool.tile([128, NF * nch_tot], F32, tag="trA", name="trA_t")
        bfR_t = wiR_t = None
        if nch_r:
            bfR_t = cpool.tile([128, KT, 128 * nch_r], F16, tag="bfR", name="bfR_t")
            wiR_t = cpool.tile([128, KT, 128 * nch_r], F16, tag="wiR", name="wiR_t")

        def bf_ap(kb, c):
            i = chunks_flat.index((kb, c))
            if kb == 0:
                return bf0_t[:, :, 128 * i:128 * (i + 1)]
            return bfR_t[:, :, 128 * (i - nch_0):128 * (i - nch_0 + 1)]

        def wi_ap(kb, c):
            i = chunks_flat.index((kb, c))
            if kb == 0:
                return wi0_t[:, :, 128 * i:128 * (i + 1)]
            return wiR_t[:, :, 128 * (i - nch_0):128 * (i - nch_0 + 1)]

        xbuf = [big.tile([128, KT, COLS], F16, tag=f"xb{i}", name=f"xb{i}") for i in range(2)]
        accb = big.tile([128, KT, COLS], F16, tag="acc", name="acc")
        # scat col k: rows 0:64 = out_{k-1} (scan), rows 64:128 = out_{k-2}
        # (cross-partition shifted copy); col 0 zero feeds the overlap edge
        scat = cpool.tile([128, BL, NF + 1], F16, tag="scat", name="scat")
        nc.vector.memset(scat[:, :, 0:2], 0.0)
        # uvs col NF stays zero: the sign-combine then covers all 256 cols
        uvs = cpool.tile([128, BL, NF + 1], F16, tag="uvs", name="uvs")
        nc.vector.memset(uvs[:, :, NF:NF + 1], 0.0)

        # ---- load x: one DMA per k-tile slab, alternating the two HWDGE
        # rings; the forward's a-th matmul fires as soon as slab a lands ----
        for m in range(KT):
            dma_eng = nc.sync if m % 2 == 0 else nc.scalar
            dma_eng.dma_start(xbuf[0][:, m, :], x_d[:, m, :])
        nc.gpsimd.dma_start(sgA_t[:], cons["sgA"][:])
        nc.sync.dma_start(trA_t[:], cons["trA"][:])
        if nch_r:
            nc.scalar.dma_start(bfR_t[:], cons["bfR"][:])
            nc.scalar.dma_start(wiR_t[:], cons["wiR"][:])
        for m in range(KT):
            if w_equal:
                nc.vector.tensor_copy(accb[:, m, :], xbuf[0][:, m, :])
            else:
                nc.vector.tensor_scalar_mul(accb[:, m, :], xbuf[0][:, m, :],
                                            float(wmix[0]))

        if debug_taps:
            nc.sync.dma_start(taps["t_x16"][:], xbuf[0][:])

        pending_acc = []

        def acc_update(m, t, w, cols, pool=False):
            if w_equal and pool:
                nc.gpsimd.tensor_tensor(accb[:, m, cols], accb[:, m, cols],
                                        t[:, m, cols], op=add)
            elif w_equal:
                nc.vector.tensor_tensor(accb[:, m, cols], accb[:, m, cols],
                                        t[:, m, cols], op=add)
            else:
                nc.vector.scalar_tensor_tensor(
                    accb[:, m, cols], t[:, m, cols], w, accb[:, m, cols], op0=mult, op1=add)

        def flush_acc():
            for m, t, w in pending_acc:
                acc_update(m, t, w, slice(None), pool=(m % 4 == 3))
            pending_acc.clear()

        # ---- block chain ----
        for kb in range(NBLK):
            if debug_taps and kb == 1:
                nc.sync.dma_start(taps["t_dst1"][:], xbuf[1][:])
            src = xbuf[kb % 2]
            dst = xbuf[(kb + 1) % 2]
            sizes = chunk_sizes[kb]
            nch = len(sizes)
            inv_sb = None
            if nch > 1:
                inv_sb = big.tile([128, KT, COLS], F32, tag="is", name=f"is{kb}")
            for c, nb in enumerate(sizes):
                ci = chunks_flat.index((kb, c))
                bf = bf_ap(kb, c)
                wi = wi_ap(kb, c)
                sg = sgA_t[:, ci:ci + 1]
                tr = trA_t[:, NF * ci:NF * (ci + 1)]
                gain = float(gains[kb]) / WI_SCALE

                # two independent column-group streams (batches 0-1 / 2-3):
                # PE runs fwd g0, fwd g1, inv g0, inv g1 back to back; the
                # recurrence for each group overlaps the PE work of the other,
                # and the next block's fwd g0 only waits on this block's g0
                # tanh halves
                def combine(b):
                    # batches 0-1 gate the first inverse group: their uvs copy
                    # runs on DVE so it never queues behind the previous
                    # block's tanh backlog on the scalar engine
                    eng = nc.vector.tensor_copy if b < 2 else nc.scalar.copy
                    eng(uvs[:, b, 0:NF], uv[b // 2][:, b % 2, :])
                    in1 = work.tile([128, NF], F16, tag="in1", name=f"in1_{kb}_{c}_{b}")
                    nc.vector.scalar_tensor_tensor(
                        in1[:], uvs[:, b, 1:NF + 1], sg,
                        uvs[:, b, 0:NF], op0=mult, op1=add)
                    nc.vector.tensor_tensor_scan(
                        scat[0:64, b, 1:NF + 1], tr[0:64, :], in1[0:64, :],
                        initial=0.0, op0=mult, op1=add)
                    nc.gpsimd.tensor_copy(scat[64:128, b, 1:NF + 1],
                                          scat[0:64, b, 0:NF])

                uv = [None, None]
                for g in range(2):
                    uv[g] = pmm.tile([128, 2, NF], F32, tag="uv", name=f"uv{kb}_{c}_{g}")
                    for a in range(KT):
                        nc.tensor.matmul(uv[g][:], bf[:, a, :],
                                         src[:, a, g * 512:(g + 1) * 512],
                                         start=(a == 0), stop=(a == KT - 1))
                    combine(2 * g)
                    combine(2 * g + 1)
                if debug_taps and kb == 0:
                    nc.scalar.dma_start(taps["t_scat1"][:], scat[:])
                flush_acc()
                last = kb == NBLK - 1 and c == nch - 1

                def inv_mm(g, m):
                    ip = pmi.tile([128, 512], F32, tag="ips", name=f"ip{kb}_{c}_{g}_{m}")
                    nc.tensor.matmul(ip[:], wi[:, m, :],
                                     scat[:, 2 * g:2 * g + 2, 1:NF + 1],
                                     start=True, stop=True)
                    if nch == 1:
                        return ip, dst[:, m, g * 512:(g + 1) * 512]
                    half = inv_sb[:, m, g * 512:(g + 1) * 512]
                    if c == 0:
                        nc.vector.tensor_copy(half, ip[:])
                    else:
                        nc.vector.tensor_add(half, half, ip[:])
                    return ip, half

                for m in range(KT):
                    ip0, h0 = inv_mm(0, m)
                    if nch == 1:
                        nc.scalar.activation(h0, ip0[:], Tanh, scale=gain)
                for m in range(KT):
                    ip1, h1 = inv_mm(1, m)
                    if nch == 1:
                        nc.scalar.activation(h1, ip1[:], Tanh, scale=gain)
                    elif c == nch - 1:
                        nc.scalar.activation(dst[:, m, :], inv_sb[:, m, :], Tanh,
                                             scale=gain)
                    if c == nch - 1:
                        if last:
                            # final block: accumulate and DMA this m-tile out
                            # right away so the store streams while the
                            # remaining inverse columns still compute
                            acc_update(m, dst, float(wmix[kb + 1]), slice(None))
                            dma_eng = nc.sync if m % 2 == 0 else nc.scalar
                            dma_eng.dma_start(out_d[:, m, :], accb[:, m, :])
                        else:
                            pending_acc.append((m, dst, float(wmix[kb + 1])))

        # stores already streamed out inside block 4
        pending_acc.clear()
        if debug_taps:
            nc.sync.dma_start(taps["t_acc"][:], accb[:])

    nc.compile()
    return nc


def _const_map(transfers, plans):
    bfs, wis, sgs, trs, nch0 = [], [], [], [], None
    for kb in range(NBLK):
        mats = _host_matrices(transfers[kb].astype(np.float64), plans[kb])
        if kb == 0:
            nch0 = len(mats)
        for md in mats:
            bfs.append(md["bf"])
            wis.append(md["winv"])
            sgs.append(md["sign"])
            trs.append(md["tcrep"])
    cm = {
        "bf0": np.ascontiguousarray(np.concatenate(bfs[:nch0], axis=2)),
        "wi0": np.ascontiguousarray(np.concatenate(wis[:nch0], axis=2)),
        "sgA": np.ascontiguousarray(np.concatenate(sgs, axis=1)),
        "trA": np.ascontiguousarray(np.concatenate(trs, axis=1)),
    }
    if len(bfs) > nch0:
        cm["bfR"] = np.ascontiguousarray(np.concatenate(bfs[nch0:], axis=2))
        cm["wiR"] = np.ascontiguousarray(np.concatenate(wis[nch0:], axis=2))
    return cm


def _in_maps(x, const_map):
    # host-side shard + transpose to the device layout:
    # x16t[p, m, b*NF + f] = x[b, f*1024 + m*128 + p]
    xr = np.asarray(x, dtype=np.float32).reshape(B, T).astype(np.float16)
    maps = []
    for core in range(NCORES):
        xc = xr[core * BL:(core + 1) * BL].reshape(BL, NF, KT, 128)
        m = dict(const_map)
        m["x"] = np.ascontiguousarray(xc.transpose(3, 2, 0, 1).reshape(128, KT, COLS))
        maps.append(m)
    return maps


def _out_gather(res, scale):
    outs = []
    for i in range(NCORES):
        ot = res.results[i]["out"].reshape(128, KT, BL, NF)
        oc = ot.transpose(2, 3, 1, 0).reshape(BL, 1, T)
        outs.append(oc)
    return (np.concatenate(outs, axis=0).astype(np.float32) * np.float32(scale))


def kernel(x, transfers, gains, mixer):
    transfers = np.asarray(transfers, dtype=np.float32)
    gains = np.asarray(gains, dtype=np.float64)
    mixer = np.asarray(mixer, dtype=np.float64)
    wm = np.exp(mixer - mixer.max())
    wm = wm / wm.sum()

    plans = [_plan_chunks(transfers[kb]) for kb in range(NBLK)]
    chunk_sizes = tuple(tuple(len(ch) for ch in pl) for pl in plans)
    key = (chunk_sizes, tuple(np.round(gains, 9)), tuple(np.round(wm, 9)))
    if key not in _CACHE:
        _CACHE[key] = _build(chunk_sizes, gains, wm)
    nc = _CACHE[key]

    in_maps = _in_maps(x, _const_map(transfers, plans))
    res = bass_utils.run_bass_kernel_spmd(nc, in_maps, core_ids=list(range(NCORES)))
    w_equal = bool(np.allclose(wm, wm[0], rtol=1e-7, atol=0.0))
    return _out_gather(res, wm[0] if w_equal else 1.0)
